# revision 3
# baseline (speedup 1.0000x reference)
"""PINN Navier-Stokes residual kernel for trn2 (8 cores, data parallel), v4.

13-stream Taylor jet through the 3-128x8-2 tanh MLP.  Key structure:
  - All addend scale factors (3, 2, layer-0 c0 columns) are folded into
    pre-scaled weight copies host-side (W2, W3, W1C, W8C2/3), so the
    per-layer elementwise work is exactly: 13 PSUM drains (Scalar),
    7 batched broadcast tensor_tensor products + 6 helper products
    (Vector), and a 5-op tanh chain.
  - Products are batched: one DVE op per chain factor covering all its
    consumers via a step-0 broadcast AP over a contiguous zc/zd tile.
  - A dedicated spare PSUM bank receives tiny keep-warm matmuls (paced
    by drain completions) so the PE's HAM clock gate stays at 8/8.
"""

import os
import numpy as np
from contextlib import ExitStack

import concourse.bass as bass
import concourse.bacc as bacc
import concourse.tile as tile
from concourse import mybir
from concourse.bass_utils import run_bass_kernel_spmd

F32 = mybir.dt.float32
F16 = mybir.dt.float16
OP = mybir.AluOpType
AF = mybir.ActivationFunctionType

KEEP_WARM = os.environ.get("KEEP_WARM", "1") == "1"
N_CORES = 8
N = 32768
NLOC = N // N_CORES      # 4096 points per core
BLK = 1024
NBLK = NLOC // BLK       # 4
CH = 512
NCH = BLK // CH          # 2
H = 128
PB = NLOC // H           # 32

STREAMS = ["v", "x", "y", "t", "xx", "xy", "yy", "xt", "yt",
           "xxx", "xxy", "xyy", "yyy"]
ZC6 = ("x", "y", "t", "xx", "xy", "yy")
ZD6 = ("xt", "yt", "xxx", "xxy", "xyy", "yyy")
# (stream, addend) -> (weight_variant, source_tile_key, slot)
# weight variants: 1 = W, 2 = 2W, 3 = 3W
# weight variants: "1" = W, "m2" = -2W, "m4" = -4W, "m6" = -6W
# (AEX/AEY hold s*s1*z products; the -2 of sigma'' lives in the weights)
ADDENDS = {
    "v":   [("1", "S", 0)],
    "x":   [("1", "A6C", 0)],
    "y":   [("1", "A6C", 1)],
    "t":   [("1", "A6C", 2)],
    "xx":  [("m2", "AEX", 0), ("1", "A6C", 3)],
    "xy":  [("m2", "AEX", 1), ("1", "A6C", 4)],
    "yy":  [("m2", "AEY", 0), ("1", "A6C", 5)],
    "xt":  [("m2", "AEX", 2), ("1", "AZD", 0)],
    "yt":  [("m2", "AEY", 1), ("1", "AZD", 1)],
    "xxx": [("1", "AFX", 0), ("m6", "AEX", 3), ("1", "AZD", 2)],
    "xxy": [("1", "AFX", 1), ("m2", "AEY", 2), ("m4", "AEX", 4),
            ("1", "AZD", 3)],
    "xyy": [("1", "AFY", 0), ("m2", "AEX", 5), ("m4", "AEY", 3),
            ("1", "AZD", 4)],
    "yyy": [("1", "AFY", 1), ("m6", "AEY", 4), ("1", "AZD", 5)],
}
VARIANTS = ("1", "m2", "m4", "m6")
# layer-1 consumes layer-0 chain tiles directly through c0-scaled W1:
# stream -> (chain source, c0 column or None)
L1_SRC = {"v": ("s", None), "x": ("s1", 0), "y": ("s1", 1), "t": ("s1", 2),
          "xx": ("s2", 3), "xy": ("s2", 4), "yy": ("s2", 5),
          "xt": ("s2", 6), "yt": ("s2", 7),
          "xxx": ("s3h", 8), "xxy": ("s3h", 9), "xyy": ("s3h", 10),
          "yyy": ("s3h", 11)}


def _build():
    nc = bacc.Bacc(None, target_bir_lowering=False)

    pts_d = nc.declare_dram_parameter("pts", [3, NLOC], F32, False)
    w0_d = nc.declare_dram_parameter("W0f", [3, H], F32, False)
    wh_d = {(li, v): nc.declare_dram_parameter(f"Wh{v}_{li}", [H, H], F16,
                                               False)
            for li in range(2, 8) for v in VARIANTS}
    w1c_d = nc.declare_dram_parameter("W1C", [H, 13 * H], F16, False)
    b_d = {li: nc.declare_dram_parameter(f"bb{li}", [H, 1], F32, False)
           for li in range(0, 8)}
    w8c_d = {v: nc.declare_dram_parameter(f"W8C{v}", [H, 16 * 13], F16, False)
             for v in VARIANTS}
    b8_d = nc.declare_dram_parameter("b8v", [H, 1], F32, False)
    lam_d = nc.declare_dram_parameter("lam", [H, 3], F32, False)
    cm23_d = nc.declare_dram_parameter("cm23", [H, 1], F32, False)
    out_d = {k: nc.declare_dram_parameter(k, [H, PB], F32, True)
             for k in ["uo", "vo", "fuo", "fvo"]}

    with tile.TileContext(nc) as tc, ExitStack() as ctx:
        cpool = ctx.enter_context(tc.tile_pool(name="consts", bufs=1))
        apool = ctx.enter_context(tc.tile_pool(name="A", bufs=1))
        zcp = ctx.enter_context(tc.tile_pool(name="zc", bufs=int(os.environ.get("ZCB","2"))))
        chain = ctx.enter_context(tc.tile_pool(name="chain", bufs=int(os.environ.get("CHB","3"))))
        misc = ctx.enter_context(tc.tile_pool(name="misc", bufs=1))
        fpool = ctx.enter_context(tc.tile_pool(name="fin", bufs=1))
        zpool = ctx.enter_context(
            tc.tile_pool(name="psum_z", bufs=3, space="PSUM"))
        z8pool = ctx.enter_context(
            tc.tile_pool(name="psum_z8", bufs=1, space="PSUM"))
        dpool = ctx.enter_context(
            tc.tile_pool(name="psum_dummy", bufs=1, space="PSUM"))

        def ctile(name, shape, dt):
            return cpool.tile(shape, dt, name=name, tag=name)

        # DMA order matters at startup: layer-0/1 params first.
        w0s = ctile("w0s", [3, H], F32)
        nc.sync.dma_start(w0s[:], w0_d[:])
        bss = {}
        for li in range(0, 8):
            bss[li] = ctile(f"bs{li}", [H, 1], F32)
            nc.sync.dma_start(bss[li][:], b_d[li][:])
        w1cs = ctile("w1cs", [H, 13 * H], F16)
        nc.sync.dma_start(w1cs[:], w1c_d[:])
        whs = {}
        for li in range(2, 8):
            for v in VARIANTS:
                whs[(li, v)] = ctile(f"whs{v}_{li}", [H, H], F16)
                nc.sync.dma_start(whs[(li, v)][:], wh_d[(li, v)][:])
        w8cs = {}
        for v in VARIANTS:
            w8cs[v] = ctile(f"w8cs{v}", [H, 16 * 13], F16)
            nc.sync.dma_start(w8cs[v][:], w8c_d[v][:])
        b8s = ctile("b8s", [H, 1], F32)
        nc.sync.dma_start(b8s[:], b8_d[:])
        lams = ctile("lams", [H, 3], F32)
        nc.sync.dma_start(lams[:], lam_d[:])
        cm23 = ctile("cm23", [H, 1], F32)
        nc.sync.dma_start(cm23[:], cm23_d[:])

        z8stage = misc.tile([16, NLOC], F16, name="z8stage", tag="z8stage")

        V, S = nc.vector, nc.scalar

        def chain_tiles():
            c = {}
            c["s"] = chain.tile([H, BLK], F16, name="cs", tag="cs")
            c["t1"] = chain.tile([H, BLK], F16, name="ct1", tag="ct1",
                                 bufs=1)
            c["s1"] = chain.tile([H, BLK], F16, name="cs1", tag="cs1")
            c["w3"] = chain.tile([H, BLK], F16, name="cw3", tag="cw3",
                                 bufs=1)
            c["s2"] = chain.tile([H, BLK], F16, name="cs2", tag="cs2")
            c["s3h"] = chain.tile([H, BLK], F16, name="cs3h", tag="cs3h")
            return c

        def chain_ops(li, ct, zt):
            S.activation(ct["s"][:], zt[:], AF.Tanh, bias=bss[li][:])
            S.activation(ct["t1"][:], ct["s"][:], AF.Square)
            V.tensor_scalar(ct["s1"][:], ct["t1"][:], -1.0, 1.0,
                            OP.mult, OP.add)
            # s2m = s*s1 (the -2 of sigma'' lives in the m2/m4/m6 weights)
            V.tensor_tensor(ct["s2"][:], ct["s"][:], ct["s1"][:], OP.mult)
            S.activation(ct["w3"][:], ct["t1"][:], AF.Square,
                         bias=cm23[:])
            V.tensor_scalar(ct["s3h"][:], ct["w3"][:], -6.0, 2.0 / 3.0,
                            OP.mult, OP.add)

        def bprod2(out_tile, fac_ap, zt6, lo, hi):
            k = hi - lo
            dst = out_tile[:, 0:k * BLK] if lo == 0 else None
            dst = out_tile[:, 0:k * BLK].rearrange("p (s f) -> p s f", s=k)
            src1 = zt6[:, lo * BLK:hi * BLK].rearrange(
                "p (s f) -> p s f", s=k)
            src0 = fac_ap.unsqueeze(1).broadcast_to([H, k, BLK])
            V.tensor_tensor(dst, src0, src1, OP.mult)

        def bprod(out_tile, fac, zt6, lo, hi):
            # out[:, lo*BLK:hi*BLK] = fac (broadcast) * zt6[:, lo*BLK:hi*BLK]
            k = hi - lo
            dst = out_tile[:, lo * BLK:hi * BLK].rearrange(
                "p (s f) -> p s f", s=k)
            src1 = zt6[:, lo * BLK:hi * BLK].rearrange(
                "p (s f) -> p s f", s=k)
            src0 = fac[:].unsqueeze(1).broadcast_to([H, k, BLK])
            V.tensor_tensor(dst, src0, src1, OP.mult)

        def mm_addends(get_pair, zt, adds, dummy=None, dummy_rhs=None):
            for c in range(NCH):
                csl = bass.ts(c, CH)
                for j, (lhsT, rhs) in enumerate(adds):
                    nc.tensor.matmul(zt[:, csl], lhsT, rhs[:, csl],
                                     start=(j == 0),
                                     stop=(j == len(adds) - 1))

        def keep_warm(lhsT, rhs_tile, off):
            # tiny matmul into the dedicated scratch bank; paced by the
            # availability of rhs_tile (a freshly drained tile)
            nc.tensor.matmul(dummy_ps[0:16, 0:16], lhsT[:, 0:16],
                             rhs_tile[:, off:off + 16],
                             start=True, stop=True, skip_group_check=True)

        dummy_ps = dpool.tile([H, CH], F32, name="dummy", tag="dummy")

        def hidden_layer(li, A_prev, warm_w):
            """A_prev: dict stream -> list of (lhsT AP, rhs AP-tile)."""
            ct = chain_tiles()
            zc6 = zcp.tile([H, 6 * BLK], F16, name="zc6", tag="zc6")
            zd6 = zcp.tile([H, 6 * BLK], F16, name="zd6", tag="zd6")
            warm_targets = []
            hp = {}
            exy = misc.tile([H, 2 * BLK], F16, name="exy", tag="exy", bufs=2)
            pxy2 = misc.tile([H, 2 * BLK], F16, name="pxy2", tag="pxy2",
                             bufs=1)
            fxy = misc.tile([H, 2 * BLK], F16, name="fxy", tag="fxy", bufs=2)
            for si, s in enumerate(STREAMS):
                zt = zpool.tile([H, BLK], F32, name=f"z_{s}", tag="z")
                mm_addends(None, zt, A_prev[s])
                if s == "v":
                    chain_ops(li, ct, zt)
                elif s in ZC6:
                    k = ZC6.index(s)
                    S.activation(zc6[:, k * BLK:(k + 1) * BLK], zt[:],
                                 AF.Copy)
                    warm_targets.append((zc6, k * BLK))
                else:
                    k = ZD6.index(s)
                    S.activation(zd6[:, k * BLK:(k + 1) * BLK], zt[:],
                                 AF.Copy)
                    warm_targets.append((zd6, k * BLK))
            # keep-warm dummies AFTER all real matmuls of this layer: each
            # fires when its drain completes, spreading tiny PE activity
            # across the products phase so HAM never sees an idle window.
            if KEEP_WARM:
                for wt, off in warm_targets:
                    keep_warm(warm_w, wt, off)
            S.activation(pxy2[:, 0:BLK], zc6[:, 0:BLK], AF.Square)
            S.activation(pxy2[:, BLK:2 * BLK], zc6[:, BLK:2 * BLK],
                         AF.Square)
            bprod(exy, ct["s2"], zc6, 0, 2)
            bprod(fxy, ct["s3h"], pxy2, 0, 2)
            hp["ex"] = exy[:, 0:BLK]
            hp["ey"] = exy[:, BLK:2 * BLK]
            hp["fx"] = fxy[:, 0:BLK]
            hp["fy"] = fxy[:, BLK:2 * BLK]
            # batched addend products
            A6C = apool.tile([H, 6 * BLK], F16, name="A6C", tag="A6C")
            AEX = apool.tile([H, 6 * BLK], F16, name="AEX", tag="AEX")
            AEY = apool.tile([H, 5 * BLK], F16, name="AEY", tag="AEY")
            AFX = apool.tile([H, 2 * BLK], F16, name="AFX", tag="AFX")
            AFY = apool.tile([H, 2 * BLK], F16, name="AFY", tag="AFY")
            AZD = apool.tile([H, 6 * BLK], F16, name="AZD", tag="AZD")
            bprod(A6C, ct["s1"], zc6, 0, 6)
            bprod2(AEX, hp["ex"], zc6, 0, 6)
            # AEY covers zc6 slots 1..5 -> own slots 0..4
            dst = AEY[:].rearrange("p (s f) -> p s f", s=5)
            src1 = zc6[:, BLK:6 * BLK].rearrange("p (s f) -> p s f", s=5)
            V.tensor_tensor(dst, hp["ey"].unsqueeze(1).broadcast_to(
                [H, 5, BLK]), src1, OP.mult)
            bprod2(AFX, hp["fx"], zc6, 0, 2)
            bprod2(AFY, hp["fy"], zc6, 0, 2)
            # AZD split in two for earlier availability of xt/yt
            bprod(AZD, ct["s1"], zd6, 0, 2)
            bprod(AZD, ct["s1"], zd6, 2, 6)
            if KEEP_WARM:
                keep_warm(warm_w, A6C, 0)
                keep_warm(warm_w, AZD, 0)
            tiles = {"S": ct["s"], "A6C": A6C, "AEX": AEX, "AEY": AEY,
                     "AFX": AFX, "AFY": AFY, "AZD": AZD}

            def ap_of(src, slot):
                t = tiles[src]
                if src == "S":
                    return t[:]
                return t[:, slot * BLK:(slot + 1) * BLK]

            W = {v: whs[(li + 1, v)] if li < 7 else w8cs[v]
                 for v in VARIANTS}
            A_new = {}
            for s, adds in ADDENDS.items():
                if li < 7:
                    A_new[s] = [(W[v][:], ap_of(src, slot))
                                for v, src, slot in adds]
                else:
                    si = STREAMS.index(s)
                    A_new[s] = [(W[v][:, 16 * si:16 * si + 16],
                                 ap_of(src, slot))
                                for v, src, slot in adds]
            return A_new

        def layer0(blk):
            ptsb = misc.tile([3, BLK], F32, name="ptsb", tag="ptsb", bufs=2)
            nc.sync.dma_start(ptsb[:], pts_d[:, bass.ts(blk, BLK)])
            ct = chain_tiles()
            zt = zpool.tile([H, BLK], F32, name="z0", tag="z")
            for c in range(NCH):
                csl = bass.ts(c, CH)
                nc.tensor.matmul(zt[:, csl], w0s[:], ptsb[:, csl],
                                 start=True, stop=True)
            chain_ops(0, ct, zt)
            A_new = {}
            for s in STREAMS:
                cn, col = L1_SRC[s]
                wsl = w1cs[:, 0 * H:H] if col is None else \
                    w1cs[:, (col + 1) * H:(col + 2) * H]
                A_new[s] = [(wsl, ct[cn])]
            return A_new

        def layer8(blk, A_prev):
            for c in range(NCH):
                csl = bass.ts(c, CH)
                z8 = z8pool.tile([16, CH], F32, name="z8", tag="z8")
                total = sum(len(v) for v in A_prev.values())
                k = 0
                for s in STREAMS:
                    for lhsT, rhs in A_prev[s]:
                        nc.tensor.matmul(z8[:], lhsT, rhs[:, csl],
                                         start=(k == 0),
                                         stop=(k == total - 1))
                        k += 1
                S.activation(z8stage[:, bass.ts(blk * NCH + c, CH)],
                             z8[:], AF.Copy)

        def ft(name, dt=F16):
            return fpool.tile([H, PB], dt, name=name, tag=name)

        Z = {s: ft(f"Z_{s}") for s in STREAMS}
        PBB = PB // NBLK          # final-tile cols per block

        for blk in range(NBLK):
            A = layer0(blk)
            for li in range(1, 8):
                A = hidden_layer(li, A, warm_w=w1cs)
            layer8(blk, A)
            # stream this block's psi-jet values into the final tiles
            # now.  Point p maps to Z[p // PB, p % PB], so block blk's
            # 1024 points are ROWS [32*blk, 32*blk+32) of the final tiles.
            for si, s in enumerate(STREAMS):
                nc.gpsimd.dma_start(
                    Z[s][32 * blk:32 * (blk + 1), :],
                    z8stage[si:si + 1, bass.ts(blk, BLK)])

        # ---------------- final fp32 jet -> outputs ----------------

        def tt(name, a, b, op=OP.mult):
            o = ft(name)
            V.tensor_tensor(o[:], a[:], b[:], op)
            return o

        def stt(name, a, sc, b, op0=OP.mult, op1=OP.mult):
            o = ft(name)
            V.scalar_tensor_tensor(o[:], a[:], sc, b[:], op0, op1)
            return o

        s8 = ft("s8")
        S.activation(s8[:], Z["v"][:], AF.Tanh, bias=b8s[:])
        t18 = ft("t18")
        S.activation(t18[:], s8[:], AF.Square)
        s18 = ft("s18")
        S.activation(s18[:], t18[:], AF.Copy, bias=1.0, scale=-1.0)
        w38 = ft("w38")
        S.activation(w38[:], t18[:], AF.Copy, bias=-2.0, scale=6.0)
        s28 = stt("s28", s8, -2.0, s18)
        s38 = tt("s38", w38, s18)
        e8x = tt("e8x", s28, Z["x"])
        e8y = tt("e8y", s28, Z["y"])
        p8xx = tt("p8xx", Z["x"], Z["x"])
        p8yy = tt("p8yy", Z["y"], Z["y"])
        f8x = tt("f8x", s38, p8xx)
        f8y = tt("f8y", s38, p8yy)

        u = ft("u", F32)
        V.tensor_tensor(u[:], s18[:], Z["y"][:], OP.mult)
        vv = ft("vv", F32)
        V.scalar_tensor_tensor(vv[:], s18[:], -1.0, Z["x"][:],
                               OP.mult, OP.mult)

        def second(name, Ea, Zb, Zab):
            a1 = tt(name + "_a", Ea, Zb)
            a2 = tt(name + "_b", s18, Zab)
            return tt(name, a1, a2, OP.add)

        p_xx = second("p_xx", e8x, Z["x"], Z["xx"])
        p_xy = second("p_xy", e8x, Z["y"], Z["xy"])
        p_yy = second("p_yy", e8y, Z["y"], Z["yy"])
        p_xt = second("p_xt", e8x, Z["t"], Z["xt"])
        p_yt = second("p_yt", e8y, Z["t"], Z["yt"])

        def third3(name, Fa, Za, Ea, Zaa, Zddd):
            a1 = tt(name + "_a", Fa, Za)
            a2 = stt(name + "_b", Ea, 3.0, Zaa)
            a3 = tt(name + "_c", s18, Zddd)
            a12 = tt(name + "_ab", a1, a2, OP.add)
            return tt(name, a12, a3, OP.add)

        p_xxx = third3("p_xxx", f8x, Z["x"], e8x, Z["xx"], Z["xxx"])
        p_yyy = third3("p_yyy", f8y, Z["y"], e8y, Z["yy"], Z["yyy"])

        def third_m(name, Fa, Zb, Eb, Zaa, Ea, Zab, Zddd):
            a1 = tt(name + "_a", Fa, Zb)
            a2 = tt(name + "_b", Eb, Zaa)
            a3 = stt(name + "_c", Ea, 2.0, Zab)
            a4 = tt(name + "_d", s18, Zddd)
            a12 = tt(name + "_ab", a1, a2, OP.add)
            a34 = tt(name + "_cd", a3, a4, OP.add)
            return tt(name, a12, a34, OP.add)

        p_xxy = third_m("p_xxy", f8x, Z["y"], e8y, Z["xx"], e8x, Z["xy"],
                        Z["xxy"])
        p_xyy = third_m("p_xyy", f8y, Z["x"], e8x, Z["yy"], e8y, Z["xy"],
                        Z["xyy"])

        fu_a = tt("fu_a", u, p_xy)
        fu_b = tt("fu_b", vv, p_yy)
        fu_ab = tt("fu_ab", fu_a, fu_b, OP.add)
        fu_l = stt("fu_l", fu_ab, lams[:, 0:1], p_yt, OP.mult, OP.add)
        fu_c = tt("fu_c", p_xxy, p_yyy, OP.add)
        f_u = ft("f_u", F32)
        V.scalar_tensor_tensor(f_u[:], fu_c[:], lams[:, 1:2], fu_l[:],
                               OP.mult, OP.add)
        fv_a = tt("fv_a", u, p_xx)
        fv_b = tt("fv_b", vv, p_xy)
        fv_ab = tt("fv_ab", fv_a, fv_b, OP.add)
        fv_l = stt("fv_l", fv_ab, lams[:, 0:1], p_xt, OP.mult, OP.add)
        fv_c = tt("fv_c", p_xxx, p_xyy, OP.add)
        f_v = ft("f_v", F32)
        V.scalar_tensor_tensor(f_v[:], fv_c[:], lams[:, 2:3], fv_l[:],
                               OP.mult, OP.subtract)

        nc.sync.dma_start(out_d["uo"][:], u[:])
        nc.sync.dma_start(out_d["vo"][:], vv[:])
        nc.sync.dma_start(out_d["fuo"][:], f_u[:])
        nc.sync.dma_start(out_d["fvo"][:], f_v[:])

    return nc


_CACHE = {}


def _get_nc():
    if "nc" not in _CACHE:
        nc = _build()
        nc.finalize()
        _CACHE["nc"] = nc
    return _CACHE["nc"]


def kernel(**inputs):
    nc = _get_nc()
    f32 = np.float32
    f16 = np.float16
    x = np.asarray(inputs["x"], f32)[:, 0]
    y = np.asarray(inputs["y"], f32)[:, 0]
    t = np.asarray(inputs["t"], f32)[:, 0]
    pts = np.ascontiguousarray(np.stack([x, y, t], 0))          # [3, N]
    W0 = np.asarray(inputs["W0"], f32)
    cx, cy, ct = W0[0], W0[1], W0[2]
    c0 = np.stack(
        [cx, cy, ct,
         cx * cx, cx * cy, cy * cy, cx * ct, cy * ct,
         cx ** 3, cx * cx * cy, cx * cy * cy, cy ** 3], 1).astype(f32)
    W1 = np.asarray(inputs["W1"], f32)
    # W1C: [plain W1 | W1*diag(c0_col) for the 12 derivative streams]
    w1c = np.zeros([H, 13 * H], f32)
    w1c[:, 0:H] = W1
    for col in range(12):
        sc = -2.0 if 3 <= col <= 7 else 1.0
        w1c[:, (col + 1) * H:(col + 2) * H] = sc * W1 * c0[:, col:col + 1]
    w8 = np.asarray(inputs["W8"], f32)[:, 0]
    lam1 = f32(np.asarray(inputs["lam1"]).reshape(-1)[0])
    lam2 = f32(np.asarray(inputs["lam2"]).reshape(-1)[0])
    shared = {
        "W0f": np.ascontiguousarray(W0),
        "W1C": w1c.astype(f16),
        "b8v": np.full([H, 1], np.asarray(inputs["b8"]).reshape(-1)[0], f32),
        "lam": np.tile(np.array([[lam1, -lam2, lam2]], f32), (H, 1)),
        "cm23": np.full([H, 1], -2.0 / 3.0, f32),
    }
    VSC = {"1": 1.0, "m2": -2.0, "m4": -4.0, "m6": -6.0}
    for v, sc in VSC.items():
        W8C = np.zeros([H, 16 * 13], f16)
        for s in range(13):
            W8C[:, 16 * s + s] = (sc * w8).astype(f16)
        shared[f"W8C{v}"] = W8C
    for li in range(2, 8):
        Wl = np.asarray(inputs[f"W{li}"], f32)
        for v, sc in VSC.items():
            shared[f"Wh{v}_{li}"] = (sc * Wl).astype(f16)
    for li in range(0, 8):
        shared[f"bb{li}"] = np.asarray(
            inputs[f"b{li}"], f32).reshape(H, 1).copy()

    in_maps = []
    for c in range(N_CORES):
        m = dict(shared)
        m["pts"] = np.ascontiguousarray(pts[:, c * NLOC:(c + 1) * NLOC])
        in_maps.append(m)

    trace = bool(os.environ.get("BASS_KERNEL_TRACE"))
    tdir = os.environ.get("BASS_KERNEL_TRACE_DIR") or None
    res = run_bass_kernel_spmd(nc, in_maps, list(range(N_CORES)),
                               trace=trace, tmpdir=tdir)
    kernel.last_exec_time_ns = res.exec_time_ns
    outs = []
    for name in ["uo", "vo", "fuo", "fvo"]:
        full = np.concatenate(
            [np.asarray(res.results[c][name], f32).reshape(-1)
             for c in range(N_CORES)])
        outs.append(full[:, None])
    return tuple(outs)


kernel.last_exec_time_ns = None
